# revision 34
# baseline (speedup 1.0000x reference)
"""GAT (2-layer graph attention network) Trainium2 Bass kernel, 8-core SPMD.

Sharding: core c owns output rows i in [c*512, (c+1)*512) for BOTH layers and
computes ALL 8 heads of layer 1 for those rows (column/row-parallel instead of
head-parallel). Wins vs head-parallel: the adjacency stripe adjT[:, slice] is
loaded once and reused by all 8 heads AND layer 2 (8x less HBM traffic than
one full adj per core), and the inter-layer exchange collapses to a single
AllGather of h @ Wo_ext [4096, 66] in fp8 (no ReduceScatter at all, since
each core holds complete h rows).

Math: with s_ij = f_src[i] + f_dst[j], exp(lrelu(s)) equals, up to a per-i
factor that cancels in softmax, max(g[i], r[j]) * e1[j] with
g = exp(0.8*f_src), r = exp(-0.8*f_dst), e1 = exp(f_dst - C). So the masked
unnormalized score matrix is M[j,i] = adj[j,i] * u[j,i] against a RAW lhsT
[Wh | 1], where u = (g max r) * e1 is ONE dual-op DVE tensor_scalar (4x fp16
mode). The mask multiply of each QUAD-PAIR is one Pool op over the [0:SPOOL]
free range of both quads (2D access pattern; TT-mult is the only elementwise
op Pool's silicon accepts, at 0.42 GPSIMD efficiency) and one DVE op over
[SPOOL:2048] of both quads (2x fp16 mode), sized so both engines stay
balanced. Pool's adjacency share is stored fp8 (exact for 0/1) to cut DMA-in
bytes; the per-head normalize+ELU epilogue is emitted one head late so its
ACT-chain dependency never stalls the in-order DVE/Pool queues.

kernel(**inputs) takes full unsharded inputs, returns the full output.
"""

from contextlib import ExitStack

import numpy as np

import concourse.mybir as mybir
import concourse.tile as tile
from concourse import bacc
from concourse.bass_utils import run_bass_kernel_spmd

# Steer every activation to the one ACT table set covering all functions this
# kernel uses (Exp, Identity, Ln, Copy, Relu) so no mid-kernel table reloads.
_orig_get_tables = bacc.get_activation_tables


def _pinned_tables(arch):
    tabs = _orig_get_tables(arch)
    if "natural_log_exp_and_others" in tabs:
        return {name: (funcs if name == "natural_log_exp_and_others" else set())
                for name, funcs in tabs.items()}
    return tabs


bacc.get_activation_tables = _pinned_tables

N = 4096
F = 512
D = 64          # per-head hidden == n classes
H = 8
P = 128
NT = N // P     # 32 j tiles
SL = 512        # i columns per core
NKF = F // P    # 4 contraction tiles for x @ W
NQ = 8          # j quads (4 j-tiles each)
NPAIR = NQ // 2
C_DST = 7.0     # layer-1 exponent shift: keeps u = max(g,r)*e1 under fp16 max
N_CORES = 8
SPOOL = 768     # free-range split of each quad's mask op: Pool [0:SPOOL],
                # DVE [SPOOL:4*SL]
SDVE = 4 * SL - SPOOL

F32 = mybir.dt.float32
F16 = mybir.dt.float16
F8 = mybir.dt.float8e4
A = mybir.AluOpType
AF = mybir.ActivationFunctionType

_CACHED = {}


def build_kernel():
    nc = bacc.Bacc("TRN2", num_devices=N_CORES)

    xT = nc.dram_tensor("xT", [F, N], F16, kind="ExternalInput")
    xS = nc.dram_tensor("xS", [F, SL], F16, kind="ExternalInput")
    adjP = nc.dram_tensor("adjP", [NQ * P, SPOOL], F8, kind="ExternalInput")
    adjD = nc.dram_tensor("adjD", [NQ * P, SDVE], F16, kind="ExternalInput")
    Wext = nc.dram_tensor("Wext", [F, H * 66], F16, kind="ExternalInput")
    selD = nc.dram_tensor("selD", [H, H * P], F16, kind="ExternalInput")
    Woext = nc.dram_tensor("Woext", [F, 66], F16, kind="ExternalInput")
    outT = nc.dram_tensor("outT", [D, SL], F32, kind="ExternalOutput")

    with ExitStack() as ctx:
        tc = ctx.enter_context(tile.TileContext(nc))
        psum = ctx.enter_context(tc.tile_pool(name="psum", bufs=1, space="PSUM"))
        persist = ctx.enter_context(tc.tile_pool(name="persist", bufs=1))
        work = ctx.enter_context(tc.tile_pool(name="work", bufs=1))
        dram = ctx.enter_context(tc.tile_pool(name="dram", bufs=1, space="DRAM"))

        ones1 = persist.tile([1, P], F32, tag="ones1")
        nc.vector.memset(ones1[:], 1.0)
        bias_c = persist.tile([P, 1], F32, tag="bias_c")
        nc.vector.memset(bias_c[:], -C_DST)


        # ---- input DMAs (sel first: tiny, and emit_g head-of-line blocks
        # the PE queue on it). Few, large DMAs: each dispatch serializes
        # ~0.6-1.2us on SP.SEQ/HWDGE, so k-tiles are packed side by side in
        # one SBUF tile per tensor and sliced at use. ---------------------
        sel = persist.tile([H, H * P], F16, tag="sel", name="sel")
        nc.sync.dma_start(out=sel[:], in_=selD[:])
        # wext before xs: the stage matmul chains need only wext + the first
        # xt quarter; xs is needed later (f_src chain)
        wext_all = persist.tile([P, NKF * 528], F16, tag="we", name="we")
        nc.sync.dma_start(
            out=wext_all[:].rearrange("p (k c) -> p k c", k=NKF),
            in_=Wext[:].rearrange("(k p) c -> p k c", k=NKF))
        xs_all = persist.tile([P, NKF * SL], F16, tag="xs", name="xs")
        nc.sync.dma_start(
            out=xs_all[:].rearrange("p (k c) -> p k c", k=NKF),
            in_=xS[:].rearrange("(k p) c -> p k c", k=NKF))

        def xsl(kf):
            return xs_all[:, kf * SL:(kf + 1) * SL]

        def wsl(kf, lo, hi, step=1):
            return wext_all[:, kf * 528 + lo:kf * 528 + hi:step]

        xt_sb = [persist.tile([P, N], F16, tag=f"xt{kf}", name=f"xt{kf}")
                 for kf in range(NKF)]
        # adjacency quad-pair tiles: Pool share fp8, DVE share fp16
        adjP_sb = [persist.tile([P, 2 * SPOOL], F8, tag=f"ajp{qq}",
                                name=f"ajp{qq}") for qq in range(NPAIR)]
        adjD_sb = [persist.tile([P, 2 * SDVE], F16, tag=f"ajd{qq}",
                                name=f"ajd{qq}") for qq in range(NPAIR)]

        def dma_adj_pair(qq):
            nc.sync.dma_start(
                out=adjP_sb[qq][:].rearrange("p (two c) -> p two c", two=2),
                in_=adjP[2 * qq * P:2 * (qq + 1) * P, :].rearrange(
                    "(two p) c -> p two c", two=2))
            nc.sync.dma_start(
                out=adjD_sb[qq][:].rearrange("p (two c) -> p two c", two=2),
                in_=adjD[2 * qq * P:2 * (qq + 1) * P, :].rearrange(
                    "(two p) c -> p two c", two=2))

        # j-tiles 0-7 first, then ALL adjacency pairs (head 0 sweeps all
        # 8 quads in ~18us; xt halves 2-3 only feed stages for heads 1-3,
        # emitted much later)
        for kf in range(NKF):
            nc.sync.dma_start(out=xt_sb[kf][:, 0:512],
                              in_=xT[kf * P:(kf + 1) * P, 0:512])
        dma_adj_pair(0)
        for kf in range(NKF):
            nc.sync.dma_start(out=xt_sb[kf][:, 512:1024],
                              in_=xT[kf * P:(kf + 1) * P, 512:1024])
        for qq in range(1, NPAIR):
            dma_adj_pair(qq)
        for kf in range(NKF):
            nc.sync.dma_start(out=xt_sb[kf][:, 1024:2048],
                              in_=xT[kf * P:(kf + 1) * P, 1024:2048])
        for kf in range(NKF):
            nc.sync.dma_start(out=xt_sb[kf][:, 2048:4096],
                              in_=xT[kf * P:(kf + 1) * P, 2048:4096])
        woext_all = persist.tile([P, NKF * 66], F16, tag="wo", name="wo")
        nc.sync.dma_start(
            out=woext_all[:].rearrange("p (k c) -> p k c", k=NKF),
            in_=Woext[:].rearrange("(k p) c -> p k c", k=NKF))

        # ---- f_src rows for all 8 heads in ONE matmul chain -------------
        fr_ps = psum.tile([H, SL], F32, tag="bank", bufs=4, name="fr")
        for kf in range(NKF):
            nc.tensor.matmul(fr_ps[:], wsl(kf, 64, 528, 66), xsl(kf),
                             start=(kf == 0), stop=(kf == NKF - 1))
        fsr = persist.tile([H, SL], F16, tag="fsr", name="fsr")
        nc.scalar.activation(fsr[:], fr_ps[:], AF.Copy)
        # sel: one-hot selector tiles, sel[k, h*128+p] = (k == h), so a K=8
        # matmul against the full fsr broadcasts row h down 128 partitions
        # without a partition-offset rhs (which BIR rejects). Host-provided.
        g_bc = [persist.tile([P, SL], F16, tag=f"g{h}", name=f"g{h}")
                for h in range(H)]

        def emit_g(h):
            bc_ps = psum.tile([P, SL], F32, tag="bank", bufs=4, name=f"gb{h}")
            nc.tensor.matmul(bc_ps[:], sel[:, h * P:(h + 1) * P], fsr[:],
                             start=True, stop=True)
            nc.scalar.activation(g_bc[h][:], bc_ps[:], AF.Exp, scale=0.8)

        # ---- stage prep: Wh_ext tiles (4 heads wide), e1/r, ones col.
        # One contiguous mega-tile for all 32 stages so the tiny per-stage
        # e1/r extractions merge into GROUPED 2D-AP ACT ops (8 stages per
        # op) -- per-op ACT overhead on 64 4-element extractions was
        # throttling the whole ramp. Copies alternate ACT/DVE. ------------
        stage_all = persist.tile([P, NT * 528], F16, tag="st", name="st")
        e1A = persist.tile([P, 4 * NT], F32, tag="e1A", name="e1A")
        rA = persist.tile([P, 4 * NT], F32, tag="rA", name="rA")
        e1B = persist.tile([P, 4 * NT], F32, tag="e1B", name="e1B")
        rB = persist.tile([P, 4 * NT], F32, tag="rB", name="rB")

        def emit_stage_mm_copy(t, qd, on_dve):
            wh_ps = psum.tile([P, 264], F32, tag="bank", bufs=4,
                              name=f"wh{t}_{qd}")
            for kf in range(NKF):
                nc.tensor.matmul(
                    wh_ps[:], xt_sb[kf][:, t * P:(t + 1) * P],
                    wsl(kf, qd * 264, (qd + 1) * 264),
                    start=(kf == 0), stop=(kf == NKF - 1))
            dst = stage_all[:, t * 528 + qd * 264:t * 528 + (qd + 1) * 264]
            if on_dve:
                nc.vector.tensor_copy(dst, wh_ps[:])
            else:
                nc.scalar.activation(dst, wh_ps[:], AF.Copy)

        def fcols(t0, nt, qd):
            """[P, nt, 4] view of the f_dst columns of stages t0..t0+nt."""
            lo = qd * 264
            return stage_all[:, t0 * 528:(t0 + nt) * 528].rearrange(
                "p (t c) -> p t c", t=nt)[:, :, lo + 65:lo + 264:66]

        def emit_extract(t0, nt, qd):
            """Grouped e1/r extraction + ones-col memset for nt stages."""
            eT, rT = (e1A, rA) if qd == 0 else (e1B, rB)
            dst_e = eT[:, 4 * t0:4 * (t0 + nt)].rearrange(
                "p (t c) -> p t c", t=nt)
            dst_r = rT[:, 4 * t0:4 * (t0 + nt)].rearrange(
                "p (t c) -> p t c", t=nt)
            nc.scalar.activation(dst_e, fcols(t0, nt, qd), AF.Exp,
                                 bias=bias_c[:])
            nc.scalar.activation(dst_r, fcols(t0, nt, qd), AF.Exp,
                                 scale=-0.8)
            lo = qd * 264
            nc.vector.memset(
                stage_all[:, t0 * 528:(t0 + nt) * 528].rearrange(
                    "p (t c) -> p t c", t=nt)[:, :, lo + 64:lo + 264:66],
                1.0)

        emit_g(0)
        for grp in range(4):
            for t in range(8 * grp, 8 * grp + 8):
                emit_stage_mm_copy(t, 0, on_dve=(t % 4 == 3))
            emit_extract(8 * grp, 8, 0)

        def lhst(h, jt):
            lo = jt * 528 + (264 if h >= 4 else 0) + (h % 4) * 66
            return stage_all[:, lo:lo + 65]

        def scal(h, jt):
            eT, rT = (e1A, rA) if h < 4 else (e1B, rB)
            c = 4 * jt + (h % 4)
            return rT[:, c:c + 1], eT[:, c:c + 1]

        def r3(ap):
            return ap.rearrange("p (two c) -> p two c", two=2)

        SP2 = 768   # L2 split: no per-head epilogue work on DVE there, so
                    # DVE can absorb more of the mask; Pool drops to [0:SP2]
                    # and DVE also covers the fp8 sliver [SP2:SPOOL]

        def emit_masked_pair(pfx, g_ap, rk, ek, qq, l2=False):
            """One quad-pair's scores: per QUAD, 2 TSPs + a Pool mask op
            over [0:SPOOL], then 2 TSPs + a DVE mask op over [SPOOL:2048].
            Fine per-quad granularity keeps PE fed (pair-sized ops stall
            the matmul chain and trip the p-state ramp). rk/ek: [two][k] ->
            [P,1] scalar APs. Returns the mq pair tile."""
            uq = work.tile([P, 8 * SL], F16, tag="uq", bufs=3,
                           name=f"uq{pfx}")
            mq = work.tile([P, 8 * SL], F16, tag="mq", bufs=3,
                           name=f"mq{pfx}")
            spool = SP2 if l2 else SPOOL
            for two in range(2):
                off = two * 4 * SL
                for k in (0, 1):
                    nc.vector.tensor_scalar(
                        uq[:, off + k * SL:off + (k + 1) * SL], g_ap,
                        rk[two][k], ek[two][k], A.max, A.mult)
                nc.gpsimd.tensor_tensor(
                    mq[:, off:off + spool], uq[:, off:off + spool],
                    adjP_sb[qq][:, two * SPOOL:two * SPOOL + spool], A.mult)
            for two in range(2):
                off = two * 4 * SL
                for k in (2, 3):
                    nc.vector.tensor_scalar(
                        uq[:, off + k * SL:off + (k + 1) * SL], g_ap,
                        rk[two][k], ek[two][k], A.max, A.mult)
                if l2 and SPOOL > SP2:
                    # fp8 sliver [SP2:SPOOL] (1x mode, tiny) + fp16 rest
                    nc.vector.tensor_tensor(
                        mq[:, off + SP2:off + SPOOL],
                        uq[:, off + SP2:off + SPOOL],
                        adjP_sb[qq][:, two * SPOOL + SP2:
                                     (two + 1) * SPOOL], A.mult)
                nc.vector.tensor_tensor(
                    mq[:, off + SPOOL:off + 4 * SL],
                    uq[:, off + SPOOL:off + 4 * SL],
                    adjD_sb[qq][:, two * SDVE:(two + 1) * SDVE], A.mult)
            return mq

        # DVE-owned chunks (k=2,3) first: their mask halves finish earlier
        MM_ORDER = ((0, 2), (0, 3), (1, 2), (1, 3),
                    (0, 0), (0, 1), (1, 0), (1, 1))

        # ---- layer-1 attention: h outer, quad-pair inner ----------------
        hT = [persist.tile([P, SL], F16, tag=f"hT{kt}", name=f"hT{kt}")
              for kt in range(NKF)]
        wh2acc = persist.tile([P, 264], F32, tag="wh2acc", name="wh2acc")
        fr2acc = persist.tile([1, SL], F32, tag="fr2acc", name="fr2acc")

        def emit_wh2_part(kt):
            o2_ps = psum.tile([P, 264], F32, tag="bank", bufs=4,
                              name=f"o2p{kt}")
            for sub in range(4):
                nc.tensor.matmul(o2_ps[:, sub * 66:(sub + 1) * 66],
                                 hT[kt][:, sub * P:(sub + 1) * P],
                                 woext_all[:, kt * 66:(kt + 1) * 66],
                                 start=True, stop=True)
            if kt == 0:
                nc.scalar.activation(wh2acc[:], o2_ps[:], AF.Copy)
            else:
                nc.vector.tensor_add(wh2acc[:], wh2acc[:], o2_ps[:])


        def make_epilogue(h, acc):
            """Normalize + ELU for head h. Emitted one head LATE (during
            head h+1's first pair) so the ACT den->ln->exp chain never
            stalls the in-order DVE/Pool queues; h==7 runs immediately
            with the faster DVE-reciprocal path (it IS the critical path
            into the AllGather)."""
            def emit():
                den_sb = work.tile([1, SL], F32, tag="den", bufs=2,
                                   name=f"den{h}")
                nc.scalar.activation(den_sb[:], acc[D:D + 1, :], AF.Copy)
                db_ps = psum.tile([D, SL], F32, tag="bank", bufs=4,
                                  name=f"dbc{h}")
                nc.tensor.matmul(db_ps[:], ones1[0:1, 0:D], den_sb[:],
                                 start=True, stop=True)
                lnb = work.tile([D, SL], F32, tag="lnb", bufs=2,
                                name=f"lnb{h}")
                nc.scalar.activation(lnb[:], db_ps[:], AF.Ln)
                # scaled-fp16 normalize: acc*2^-16 (ACT Identity applies
                # the affine scale; Copy does NOT on silicon) times 2^16/den
                # in fp16 (safe: min den ~1.9) -> DVE multiply in 2x mode
                recb = work.tile([D, SL], F16, tag="recb", bufs=2,
                                 name=f"recb{h}")
                nc.scalar.activation(recb[:], lnb[:], AF.Exp, scale=-1.0,
                                     bias=bias16[0:D, 0:1])
                acc_sb = work.tile([D, SL], F16, tag="accsb", bufs=2,
                                   name=f"accsb{h}")
                nc.scalar.activation(acc_sb[:], acc[0:D, :], AF.Identity,
                                     scale=float(2.0 ** -16))
                hsl = hT[h // 2][(h % 2) * D:(h % 2) * D + D, :]
                nc.vector.tensor_mul(hsl, acc_sb[:], recb[:])
                if h == H - 1:
                    # critical path into the gather: elu(x) =
                    # (min(exp(x),1) - 1) + max(x,0) -- h values are < ~4
                    # so exp(x) cannot overflow fp16
                    texp = work.tile([D, SL], F16, tag="texp", bufs=2,
                                     name=f"texp{h}")
                    nc.scalar.activation(texp[:], hsl, AF.Exp)
                    t1 = work.tile([D, SL], F16, tag="tlin", bufs=2,
                                   name=f"t1{h}")
                    nc.vector.tensor_scalar(t1[:], texp[:], 1.0, -1.0,
                                            A.min, A.add)
                    tl = work.tile([D, SL], F16, tag="a1", bufs=2,
                                   name=f"tl{h}")
                    nc.vector.tensor_scalar_max(tl[:], hsl, 0.0)
                    nc.vector.tensor_add(hsl, t1[:], tl[:])
                else:
                    # elu(x) = (x max 0 - 1) + exp(-relu(-x)), ACT-heavy
                    a1 = work.tile([D, SL], F16, tag="a1", bufs=2,
                                   name=f"a1{h}")
                    nc.scalar.activation(a1[:], hsl, AF.Relu, scale=-1.0)
                    texp = work.tile([D, SL], F16, tag="texp", bufs=2,
                                     name=f"texp{h}")
                    nc.scalar.activation(texp[:], a1[:], AF.Exp, scale=-1.0)
                    tlin = work.tile([D, SL], F16, tag="tlin", bufs=2,
                                     name=f"tlin{h}")
                    nc.vector.tensor_scalar(tlin[:], hsl, 0.0, -1.0, A.max,
                                            A.add)
                    nc.vector.tensor_add(hsl, texp[:], tlin[:])
            return emit

        pending = None
        for h in range(H):
            acc = psum.tile([D + 1, SL], F32, tag="acc", bufs=4,
                            name=f"acc{h}")
            for qq in range(NPAIR):
                if qq == 1 and h + 1 < H:
                    emit_g(h + 1)
                if h < 4:
                    t0 = h * 8 + 2 * qq
                    for t in (t0, t0 + 1):
                        emit_stage_mm_copy(t, 1, on_dve=False)
                    emit_extract(t0, 2, 1)
                rk, ek = [], []
                for two in range(2):
                    rr, ee = [], []
                    for k in range(4):
                        r_ap, e_ap = scal(h, 8 * qq + 4 * two + k)
                        rr.append(r_ap)
                        ee.append(e_ap)
                    rk.append(rr)
                    ek.append(ee)
                mq = emit_masked_pair(f"{h}_{qq}", g_bc[h][:], rk, ek, qq)
                for mi, (two, k) in enumerate(MM_ORDER):
                    nc.tensor.matmul(
                        acc[:], lhst(h, 8 * qq + 4 * two + k),
                        mq[:, (4 * two + k) * SL:(4 * two + k + 1) * SL],
                        start=(qq == 0 and mi == 0),
                        stop=(qq == NPAIR - 1 and mi == 7))
                if qq == 0 and pending is not None:
                    pending()
                    pending = None
                    if h % 2 == 0 and h > 1:
                        emit_wh2_part(h // 2 - 1)
            pending = make_epilogue(h, acc)
        pending()

        # ---- last wh2 part + exchange staging: the kt=3 accumulation adds
        # straight into the fp8 p2 tile (skips a separate convert copy) ----
        cc_in = dram.tile([SL, 66], F8, tag="cc_in", name="cc_in")
        cc_full = dram.tile([N, 66], F8, tag="cc_full", addr_space="Shared",
                            name="cc_full")
        p2_sb = work.tile([P, 264], F8, tag="p2", name="p2")
        o3_ps = psum.tile([P, 264], F32, tag="bank", bufs=4, name="o2p3")
        for sub in range(4):
            nc.tensor.matmul(o3_ps[:, sub * 66:(sub + 1) * 66],
                             hT[3][:, sub * P:(sub + 1) * P],
                             woext_all[:, 3 * 66:4 * 66],
                             start=True, stop=True)
        nc.vector.tensor_add(p2_sb[:], wh2acc[:], o3_ps[:])
        nc.sync.dma_start(
            out=cc_in[:].rearrange("(k p) c -> p k c", k=4),
            in_=p2_sb[:].rearrange("p (k c) -> p k c", k=4))
        nc.gpsimd.collective_compute(
            "AllGather", A.bypass, ins=[cc_in[:]], outs=[cc_full[:]],
            replica_groups=[list(range(N_CORES))])

        # f_src2 + g2 prep executes inside the collective's idle window (it
        # only feeds the layer-2 TSPs); its 1x PSUM adds used to burn
        # mid-phase DVE time. Pure reorder: hT tiles are immutable by now.
        for kt in range(NKF):
            fr2_ps = psum.tile([1, SL], F32, tag="bank", bufs=4,
                               name=f"fr2p{kt}")
            nc.tensor.matmul(fr2_ps[:],
                             woext_all[:, kt * 66 + 64:kt * 66 + 65],
                             hT[kt][:], start=True, stop=True)
            if kt == 0:
                nc.scalar.activation(fr2acc[:], fr2_ps[:], AF.Copy)
            else:
                nc.vector.tensor_add(fr2acc[:], fr2acc[:], fr2_ps[:])
        bc2_ps = psum.tile([P, SL], F32, tag="bank", bufs=4, name="gbc2")
        nc.tensor.matmul(bc2_ps[:], ones1[0:1, :], fr2acc[:], start=True,
                         stop=True)
        g2 = persist.tile([P, SL], F16, tag="g2", name="g2")
        nc.scalar.activation(g2[:], bc2_ps[:], AF.Exp, scale=0.8)

        # ---- layer 2: quarter loads (one per quad-pair); e1/r extracted
        # straight from the fp8 tile and lhsT cols converted per quarter so
        # the first TSPs never wait on the full convert. The denominator
        # ones-columns are memset BEFORE the collective (convert skips
        # them), taking that off the post-gather critical path. ----------
        cc_all = persist.tile([P, NQ * 264], F16, tag="cc_all", name="cc_all")
        cc_raw = persist.tile([P, NQ * 264], F8, tag="cc_raw", name="cc_raw")
        nc.vector.memset(cc_all[:, 64:NQ * 264:66], 1.0)
        e1x = [persist.tile([P, 8], F32, tag=f"e2_{qt}", name=f"e2_{qt}")
               for qt in range(NPAIR)]
        rx = [persist.tile([P, 8], F32, tag=f"r2_{qt}", name=f"r2_{qt}")
              for qt in range(NPAIR)]
        for qt in range(NPAIR):
            sl8 = slice(qt * 528, (qt + 1) * 528)
            nc.sync.dma_start(
                out=cc_raw[:, sl8].rearrange("p (g k c) -> p g k c", g=2,
                                             k=4),
                in_=cc_full[qt * N // 4:(qt + 1) * N // 4, :].rearrange(
                    "(g k p) c -> p g k c", g=2, k=4))
            raw8 = cc_raw[:, sl8].rearrange("p (g c) -> p g c", g=8)
            nc.scalar.activation(e1x[qt][:], raw8[:, :, 65:66], AF.Exp)
            nc.scalar.activation(rx[qt][:], raw8[:, :, 65:66], AF.Exp,
                                 scale=-0.8)
            nc.scalar.activation(
                cc_all[:, sl8].rearrange("p (g c) -> p g c", g=8)[:, :, 0:64],
                raw8[:, :, 0:64], AF.Copy)

        cc_gp = [cc_all[:, g * 264:(g + 1) * 264] for g in range(NQ)]
        acc2 = psum.tile([D + 1, SL], F32, tag="acc", bufs=4, name="acc2")
        for qq in range(NPAIR):
            mq = emit_masked_pair(
                f"L2_{qq}", g2[:],
                [[rx[qq][:, 4 * two + k:4 * two + k + 1] for k in range(4)]
                 for two in range(2)],
                [[e1x[qq][:, 4 * two + k:4 * two + k + 1] for k in range(4)]
                 for two in range(2)], qq, l2=True)
            for mi, (two, k) in enumerate(MM_ORDER):
                nc.tensor.matmul(
                    acc2[:],
                    cc_gp[2 * qq + two][:, k * 66:k * 66 + 65],
                    mq[:, (4 * two + k) * SL:(4 * two + k + 1) * SL],
                    start=(qq == 0 and mi == 0),
                    stop=(qq == NPAIR - 1 and mi == 7))
        # final normalize + ELU pipelined in two i-halves so ACT and DVE
        # overlap. Scaled-fp16 normalize (Identity applies the affine
        # scale) gets the DVE ops into 2x/4x mode; L2 denominators are
        # O(100+) so 2^16/den is far inside fp16 range. fin is assembled
        # in fp32 only at the last add.
        o2 = persist.tile([D, SL], F16, tag="o2", name="o2")
        fin = persist.tile([D, SL], F32, tag="fin", name="fin")
        for hf in range(2):
            sl2 = slice(hf * SL // 2, (hf + 1) * SL // 2)
            den2 = work.tile([1, SL // 2], F32, tag="den", bufs=2,
                             name=f"den2_{hf}")
            nc.scalar.activation(den2[:], acc2[D:D + 1, sl2], AF.Copy)
            db2_ps = psum.tile([D, SL // 2], F32, tag="bank", bufs=4,
                               name=f"dbc2_{hf}")
            nc.tensor.matmul(db2_ps[:], ones1[0:1, 0:D], den2[:],
                             start=True, stop=True)
            ln2 = work.tile([D, SL // 2], F32, tag="lnb", bufs=2,
                            name=f"ln2_{hf}")
            nc.scalar.activation(ln2[:], db2_ps[:], AF.Ln)
            rec2 = work.tile([D, SL // 2], F16, tag="recb", bufs=2,
                             name=f"rec2_{hf}")
            nc.scalar.activation(rec2[:], ln2[:], AF.Exp, scale=-1.0,
                                 bias=bias16[0:D, 0:1])
            ac2_sb = work.tile([D, SL // 2], F16, tag="accsb", bufs=2,
                               name=f"ac2sb{hf}")
            nc.scalar.activation(ac2_sb[:], acc2[0:D, sl2], AF.Identity,
                                 scale=float(2.0 ** -16))
            nc.vector.tensor_mul(o2[:, sl2], ac2_sb[:], rec2[:])
            # elu(y) = (min(exp(y),1) - 1) + max(y,0); y < ~4 so exp safe
            t2exp = work.tile([D, SL // 2], F16, tag="t2exp", bufs=2,
                              name=f"t2exp{hf}")
            nc.scalar.activation(t2exp[:], o2[:, sl2], AF.Exp)
            t2a = work.tile([D, SL // 2], F16, tag="t2lin", bufs=2,
                            name=f"t2a{hf}")
            nc.vector.tensor_scalar(t2a[:], t2exp[:], 1.0, -1.0, A.min,
                                    A.add)
            t2b = work.tile([D, SL // 2], F16, tag="a2", bufs=2,
                            name=f"t2b{hf}")
            nc.vector.tensor_scalar_max(t2b[:], o2[:, sl2], 0.0)
            nc.vector.tensor_add(fin[:, sl2], t2a[:], t2b[:])
            nc.sync.dma_start(out=outT[:, sl2], in_=fin[:, sl2])

    nc.compile()
    return nc


# ---------------------------------------------------------------------------
# host-side driver
# ---------------------------------------------------------------------------

def _prep_inputs(x, adj, W, a, Wo, ao):
    xT16 = np.ascontiguousarray(x.T.astype(np.float16))
    adjT = adj.T
    wext = np.empty((F, H * 66), np.float32)
    for h in range(H):
        wext[:, h * 66:h * 66 + D] = W[h]
        wext[:, h * 66 + D] = W[h] @ a[h, :D]
        wext[:, h * 66 + D + 1] = W[h] @ a[h, D:]
    wext = wext.astype(np.float16)
    woext = np.concatenate(
        [Wo, (Wo @ ao[:D])[:, None], (Wo @ ao[D:])[:, None]],
        axis=1).astype(np.float16)

    from ml_dtypes import float8_e4m3fn as f8dt

    in_maps = []
    for c in range(N_CORES):
        sl = slice(c * SL, (c + 1) * SL)
        adjq = np.empty((NQ * P, 4 * SL), np.float32)
        for q in range(NQ):
            for k in range(4):
                jt = 4 * q + k
                adjq[q * P:(q + 1) * P, k * SL:(k + 1) * SL] = \
                    adjT[jt * P:(jt + 1) * P, sl]
        selDh = np.zeros((H, H * P), np.float16)
        for h in range(H):
            selDh[h, h * P:(h + 1) * P] = 1.0
        in_maps.append({
            "xT": xT16,
            "xS": np.ascontiguousarray(xT16[:, sl]),
            "adjP": adjq[:, :SPOOL].astype(f8dt),
            "adjD": adjq[:, SPOOL:].astype(np.float16),
            "Wext": wext,
            "Woext": woext,
            "selD": selDh,
        })
    return in_maps


def kernel(x, adj, W, a, Wo, ao, cfg):
    x = np.asarray(x, np.float32)
    adj = np.asarray(adj, np.float32)
    W = np.asarray(W, np.float32)
    a = np.asarray(a, np.float32)
    Wo = np.asarray(Wo, np.float32)
    ao = np.asarray(ao, np.float32)

    in_maps = _prep_inputs(x, adj, W, a, Wo, ao)
    if _CACHED.get("nc") is None:
        _CACHED["nc"] = build_kernel()
    res = run_bass_kernel_spmd(_CACHED["nc"], in_maps,
                               core_ids=list(range(N_CORES)))
    out = np.empty((N, D), np.float32)
    for c in range(N_CORES):
        out[c * SL:(c + 1) * SL, :] = res.results[c]["outT"].T
    return out


if __name__ == "__main__":
    import reference as ref_mod
    inputs = {k: np.asarray(v) for k, v in ref_mod.setup_inputs().items()}
    expected = np.asarray(ref_mod.reference(**ref_mod.setup_inputs()))
    got = kernel(**inputs)
    err = np.abs(got - expected).max() / np.abs(expected).max()
    print("rel err:", err)


# revision 36
# speedup vs baseline: 1.0009x; 1.0009x over previous
"""GAT (2-layer graph attention network) Trainium2 Bass kernel, 8-core SPMD.

Sharding: core c owns output rows i in [c*512, (c+1)*512) for BOTH layers and
computes ALL 8 heads of layer 1 for those rows (column/row-parallel instead of
head-parallel). Wins vs head-parallel: the adjacency stripe adjT[:, slice] is
loaded once and reused by all 8 heads AND layer 2 (8x less HBM traffic than
one full adj per core), and the inter-layer exchange collapses to a single
AllGather of h @ Wo_ext [4096, 66] in fp8 (no ReduceScatter at all, since
each core holds complete h rows).

Math: with s_ij = f_src[i] + f_dst[j], exp(lrelu(s)) equals, up to a per-i
factor that cancels in softmax, max(g[i], r[j]) * e1[j] with
g = exp(0.8*f_src), r = exp(-0.8*f_dst), e1 = exp(f_dst - C). So the masked
unnormalized score matrix is M[j,i] = adj[j,i] * u[j,i] against a RAW lhsT
[Wh | 1], where u = (g max r) * e1 is ONE dual-op DVE tensor_scalar (4x fp16
mode). The mask multiply of each QUAD-PAIR is one Pool op over the [0:SPOOL]
free range of both quads (2D access pattern; TT-mult is the only elementwise
op Pool's silicon accepts, at 0.42 GPSIMD efficiency) and one DVE op over
[SPOOL:2048] of both quads (2x fp16 mode), sized so both engines stay
balanced. Pool's adjacency share is stored fp8 (exact for 0/1) to cut DMA-in
bytes; the per-head normalize+ELU epilogue is emitted one head late so its
ACT-chain dependency never stalls the in-order DVE/Pool queues.

kernel(**inputs) takes full unsharded inputs, returns the full output.
"""

from contextlib import ExitStack

import numpy as np

import concourse.mybir as mybir
import concourse.tile as tile
from concourse import bacc
from concourse.bass_utils import run_bass_kernel_spmd

# Steer every activation to the one ACT table set covering all functions this
# kernel uses (Exp, Identity, Ln, Copy, Relu) so no mid-kernel table reloads.
_orig_get_tables = bacc.get_activation_tables


def _pinned_tables(arch):
    tabs = _orig_get_tables(arch)
    if "natural_log_exp_and_others" in tabs:
        return {name: (funcs if name == "natural_log_exp_and_others" else set())
                for name, funcs in tabs.items()}
    return tabs


bacc.get_activation_tables = _pinned_tables

N = 4096
F = 512
D = 64          # per-head hidden == n classes
H = 8
P = 128
NT = N // P     # 32 j tiles
SL = 512        # i columns per core
NKF = F // P    # 4 contraction tiles for x @ W
NQ = 8          # j quads (4 j-tiles each)
NPAIR = NQ // 2
C_DST = 7.0     # layer-1 exponent shift: keeps u = max(g,r)*e1 under fp16 max
N_CORES = 8
SPOOL = 776     # free-range split of each quad's mask op: Pool [0:SPOOL],
                # DVE [SPOOL:4*SL]
SDVE = 4 * SL - SPOOL

F32 = mybir.dt.float32
F16 = mybir.dt.float16
F8 = mybir.dt.float8e4
A = mybir.AluOpType
AF = mybir.ActivationFunctionType

_CACHED = {}


def build_kernel():
    nc = bacc.Bacc("TRN2", num_devices=N_CORES)

    xT = nc.dram_tensor("xT", [F, N], F16, kind="ExternalInput")
    xS = nc.dram_tensor("xS", [F, SL], F16, kind="ExternalInput")
    adjP = nc.dram_tensor("adjP", [NQ * P, SPOOL], F8, kind="ExternalInput")
    adjD = nc.dram_tensor("adjD", [NQ * P, SDVE], F16, kind="ExternalInput")
    Wext = nc.dram_tensor("Wext", [F, H * 66], F16, kind="ExternalInput")
    selD = nc.dram_tensor("selD", [H, H * P], F16, kind="ExternalInput")
    Woext = nc.dram_tensor("Woext", [F, 66], F16, kind="ExternalInput")
    outT = nc.dram_tensor("outT", [D, SL], F32, kind="ExternalOutput")

    with ExitStack() as ctx:
        tc = ctx.enter_context(tile.TileContext(nc))
        psum = ctx.enter_context(tc.tile_pool(name="psum", bufs=1, space="PSUM"))
        persist = ctx.enter_context(tc.tile_pool(name="persist", bufs=1))
        work = ctx.enter_context(tc.tile_pool(name="work", bufs=1))
        dram = ctx.enter_context(tc.tile_pool(name="dram", bufs=1, space="DRAM"))

        ones1 = persist.tile([1, P], F32, tag="ones1")
        nc.vector.memset(ones1[:], 1.0)
        bias_c = persist.tile([P, 1], F32, tag="bias_c")
        nc.vector.memset(bias_c[:], -C_DST)


        # ---- input DMAs (sel first: tiny, and emit_g head-of-line blocks
        # the PE queue on it). Few, large DMAs: each dispatch serializes
        # ~0.6-1.2us on SP.SEQ/HWDGE, so k-tiles are packed side by side in
        # one SBUF tile per tensor and sliced at use. ---------------------
        sel = persist.tile([H, H * P], F16, tag="sel", name="sel")
        nc.sync.dma_start(out=sel[:], in_=selD[:])
        # wext before xs: the stage matmul chains need only wext + the first
        # xt quarter; xs is needed later (f_src chain)
        wext_all = persist.tile([P, NKF * 528], F16, tag="we", name="we")
        nc.sync.dma_start(
            out=wext_all[:].rearrange("p (k c) -> p k c", k=NKF),
            in_=Wext[:].rearrange("(k p) c -> p k c", k=NKF))
        xs_all = persist.tile([P, NKF * SL], F16, tag="xs", name="xs")
        nc.sync.dma_start(
            out=xs_all[:].rearrange("p (k c) -> p k c", k=NKF),
            in_=xS[:].rearrange("(k p) c -> p k c", k=NKF))

        def xsl(kf):
            return xs_all[:, kf * SL:(kf + 1) * SL]

        def wsl(kf, lo, hi, step=1):
            return wext_all[:, kf * 528 + lo:kf * 528 + hi:step]

        xt_sb = [persist.tile([P, N], F16, tag=f"xt{kf}", name=f"xt{kf}")
                 for kf in range(NKF)]
        # adjacency quad-pair tiles: Pool share fp8, DVE share fp16
        adjP_sb = [persist.tile([P, 2 * SPOOL], F8, tag=f"ajp{qq}",
                                name=f"ajp{qq}") for qq in range(NPAIR)]
        adjD_sb = [persist.tile([P, 2 * SDVE], F16, tag=f"ajd{qq}",
                                name=f"ajd{qq}") for qq in range(NPAIR)]

        def dma_adj_pair(qq):
            nc.sync.dma_start(
                out=adjP_sb[qq][:].rearrange("p (two c) -> p two c", two=2),
                in_=adjP[2 * qq * P:2 * (qq + 1) * P, :].rearrange(
                    "(two p) c -> p two c", two=2))
            nc.sync.dma_start(
                out=adjD_sb[qq][:].rearrange("p (two c) -> p two c", two=2),
                in_=adjD[2 * qq * P:2 * (qq + 1) * P, :].rearrange(
                    "(two p) c -> p two c", two=2))

        # j-tiles 0-7 first, then ALL adjacency pairs (head 0 sweeps all
        # 8 quads in ~18us; xt halves 2-3 only feed stages for heads 1-3,
        # emitted much later)
        for kf in range(NKF):
            nc.sync.dma_start(out=xt_sb[kf][:, 0:512],
                              in_=xT[kf * P:(kf + 1) * P, 0:512])
        dma_adj_pair(0)
        for kf in range(NKF):
            nc.sync.dma_start(out=xt_sb[kf][:, 512:1024],
                              in_=xT[kf * P:(kf + 1) * P, 512:1024])
        for qq in range(1, NPAIR):
            dma_adj_pair(qq)
        for kf in range(NKF):
            nc.sync.dma_start(out=xt_sb[kf][:, 1024:2048],
                              in_=xT[kf * P:(kf + 1) * P, 1024:2048])
        for kf in range(NKF):
            nc.sync.dma_start(out=xt_sb[kf][:, 2048:4096],
                              in_=xT[kf * P:(kf + 1) * P, 2048:4096])
        woext_all = persist.tile([P, NKF * 66], F16, tag="wo", name="wo")
        nc.sync.dma_start(
            out=woext_all[:].rearrange("p (k c) -> p k c", k=NKF),
            in_=Woext[:].rearrange("(k p) c -> p k c", k=NKF))

        # ---- f_src rows for all 8 heads in ONE matmul chain -------------
        fr_ps = psum.tile([H, SL], F32, tag="bank", bufs=4, name="fr")
        for kf in range(NKF):
            nc.tensor.matmul(fr_ps[:], wsl(kf, 64, 528, 66), xsl(kf),
                             start=(kf == 0), stop=(kf == NKF - 1))
        fsr = persist.tile([H, SL], F16, tag="fsr", name="fsr")
        nc.scalar.activation(fsr[:], fr_ps[:], AF.Copy)
        # sel: one-hot selector tiles, sel[k, h*128+p] = (k == h), so a K=8
        # matmul against the full fsr broadcasts row h down 128 partitions
        # without a partition-offset rhs (which BIR rejects). Host-provided.
        g_bc = [persist.tile([P, SL], F16, tag=f"g{h}", name=f"g{h}")
                for h in range(H)]

        def emit_g(h):
            bc_ps = psum.tile([P, SL], F32, tag="bank", bufs=4, name=f"gb{h}")
            nc.tensor.matmul(bc_ps[:], sel[:, h * P:(h + 1) * P], fsr[:],
                             start=True, stop=True)
            nc.scalar.activation(g_bc[h][:], bc_ps[:], AF.Exp, scale=0.8)

        # ---- stage prep: Wh_ext tiles (4 heads wide), e1/r, ones col.
        # One contiguous mega-tile for all 32 stages so the tiny per-stage
        # e1/r extractions merge into GROUPED 2D-AP ACT ops (8 stages per
        # op) -- per-op ACT overhead on 64 4-element extractions was
        # throttling the whole ramp. Copies alternate ACT/DVE. ------------
        stage_all = persist.tile([P, NT * 528], F16, tag="st", name="st")
        e1A = persist.tile([P, 4 * NT], F32, tag="e1A", name="e1A")
        rA = persist.tile([P, 4 * NT], F32, tag="rA", name="rA")
        e1B = persist.tile([P, 4 * NT], F32, tag="e1B", name="e1B")
        rB = persist.tile([P, 4 * NT], F32, tag="rB", name="rB")

        def emit_stage_mm_copy(t, qd, on_dve):
            wh_ps = psum.tile([P, 264], F32, tag="bank", bufs=4,
                              name=f"wh{t}_{qd}")
            for kf in range(NKF):
                nc.tensor.matmul(
                    wh_ps[:], xt_sb[kf][:, t * P:(t + 1) * P],
                    wsl(kf, qd * 264, (qd + 1) * 264),
                    start=(kf == 0), stop=(kf == NKF - 1))
            dst = stage_all[:, t * 528 + qd * 264:t * 528 + (qd + 1) * 264]
            if on_dve:
                nc.vector.tensor_copy(dst, wh_ps[:])
            else:
                nc.scalar.activation(dst, wh_ps[:], AF.Copy)

        def fcols(t0, nt, qd):
            """[P, nt, 4] view of the f_dst columns of stages t0..t0+nt."""
            lo = qd * 264
            return stage_all[:, t0 * 528:(t0 + nt) * 528].rearrange(
                "p (t c) -> p t c", t=nt)[:, :, lo + 65:lo + 264:66]

        def emit_extract(t0, nt, qd):
            """Grouped e1/r extraction + ones-col memset for nt stages."""
            eT, rT = (e1A, rA) if qd == 0 else (e1B, rB)
            dst_e = eT[:, 4 * t0:4 * (t0 + nt)].rearrange(
                "p (t c) -> p t c", t=nt)
            dst_r = rT[:, 4 * t0:4 * (t0 + nt)].rearrange(
                "p (t c) -> p t c", t=nt)
            nc.scalar.activation(dst_e, fcols(t0, nt, qd), AF.Exp,
                                 bias=bias_c[:])
            nc.scalar.activation(dst_r, fcols(t0, nt, qd), AF.Exp,
                                 scale=-0.8)
            lo = qd * 264
            nc.vector.memset(
                stage_all[:, t0 * 528:(t0 + nt) * 528].rearrange(
                    "p (t c) -> p t c", t=nt)[:, :, lo + 64:lo + 264:66],
                1.0)

        emit_g(0)
        for grp in range(4):
            for t in range(8 * grp, 8 * grp + 8):
                emit_stage_mm_copy(t, 0, on_dve=(t % 4 == 3))
            emit_extract(8 * grp, 8, 0)

        def lhst(h, jt):
            lo = jt * 528 + (264 if h >= 4 else 0) + (h % 4) * 66
            return stage_all[:, lo:lo + 65]

        def scal(h, jt):
            eT, rT = (e1A, rA) if h < 4 else (e1B, rB)
            c = 4 * jt + (h % 4)
            return rT[:, c:c + 1], eT[:, c:c + 1]

        def r3(ap):
            return ap.rearrange("p (two c) -> p two c", two=2)

        SP2 = 768   # L2 split: no per-head epilogue work on DVE there, so
                    # DVE can absorb more of the mask; Pool drops to [0:SP2]
                    # and DVE also covers the fp8 sliver [SP2:SPOOL]

        def emit_masked_pair(pfx, g_ap, rk, ek, qq, l2=False):
            """One quad-pair's scores: per QUAD, 2 TSPs + a Pool mask op
            over [0:SPOOL], then 2 TSPs + a DVE mask op over [SPOOL:2048].
            Fine per-quad granularity keeps PE fed (pair-sized ops stall
            the matmul chain and trip the p-state ramp). rk/ek: [two][k] ->
            [P,1] scalar APs. Returns the mq pair tile."""
            uq = work.tile([P, 8 * SL], F16, tag="uq", bufs=3,
                           name=f"uq{pfx}")
            mq = work.tile([P, 8 * SL], F16, tag="mq", bufs=3,
                           name=f"mq{pfx}")
            spool = SP2 if l2 else SPOOL
            for two in range(2):
                off = two * 4 * SL
                for k in (0, 1):
                    nc.vector.tensor_scalar(
                        uq[:, off + k * SL:off + (k + 1) * SL], g_ap,
                        rk[two][k], ek[two][k], A.max, A.mult)
                nc.gpsimd.tensor_tensor(
                    mq[:, off:off + spool], uq[:, off:off + spool],
                    adjP_sb[qq][:, two * SPOOL:two * SPOOL + spool], A.mult)
            for two in range(2):
                off = two * 4 * SL
                for k in (2, 3):
                    nc.vector.tensor_scalar(
                        uq[:, off + k * SL:off + (k + 1) * SL], g_ap,
                        rk[two][k], ek[two][k], A.max, A.mult)
                if l2:
                    # fp8 sliver [SP2:SPOOL] (1x mode, tiny) + fp16 rest
                    nc.vector.tensor_tensor(
                        mq[:, off + SP2:off + SPOOL],
                        uq[:, off + SP2:off + SPOOL],
                        adjP_sb[qq][:, two * SPOOL + SP2:
                                     (two + 1) * SPOOL], A.mult)
                nc.vector.tensor_tensor(
                    mq[:, off + SPOOL:off + 4 * SL],
                    uq[:, off + SPOOL:off + 4 * SL],
                    adjD_sb[qq][:, two * SDVE:(two + 1) * SDVE], A.mult)
            return mq

        # DVE-owned chunks (k=2,3) first: their mask halves finish earlier
        MM_ORDER = ((0, 2), (0, 3), (1, 2), (1, 3),
                    (0, 0), (0, 1), (1, 0), (1, 1))

        # ---- layer-1 attention: h outer, quad-pair inner ----------------
        hT = [persist.tile([P, SL], F16, tag=f"hT{kt}", name=f"hT{kt}")
              for kt in range(NKF)]
        wh2acc = persist.tile([P, 264], F32, tag="wh2acc", name="wh2acc")
        fr2acc = persist.tile([1, SL], F32, tag="fr2acc", name="fr2acc")

        def emit_wh2_part(kt):
            o2_ps = psum.tile([P, 264], F32, tag="bank", bufs=4,
                              name=f"o2p{kt}")
            for sub in range(4):
                nc.tensor.matmul(o2_ps[:, sub * 66:(sub + 1) * 66],
                                 hT[kt][:, sub * P:(sub + 1) * P],
                                 woext_all[:, kt * 66:(kt + 1) * 66],
                                 start=True, stop=True)
            if kt == 0:
                nc.scalar.activation(wh2acc[:], o2_ps[:], AF.Copy)
            else:
                nc.vector.tensor_add(wh2acc[:], wh2acc[:], o2_ps[:])


        def make_epilogue(h, acc):
            """Normalize + ELU for head h. Emitted one head LATE (during
            head h+1's first pair) so the ACT den->ln->exp chain never
            stalls the in-order DVE/Pool queues; h==7 runs immediately
            with the faster DVE-reciprocal path (it IS the critical path
            into the AllGather)."""
            def emit():
                den_sb = work.tile([1, SL], F32, tag="den", bufs=2,
                                   name=f"den{h}")
                nc.scalar.activation(den_sb[:], acc[D:D + 1, :], AF.Copy)
                db_ps = psum.tile([D, SL], F32, tag="bank", bufs=4,
                                  name=f"dbc{h}")
                nc.tensor.matmul(db_ps[:], ones1[0:1, 0:D], den_sb[:],
                                 start=True, stop=True)
                lnb = work.tile([D, SL], F32, tag="lnb", bufs=2,
                                name=f"lnb{h}")
                nc.scalar.activation(lnb[:], db_ps[:], AF.Ln)
                # scaled-fp16 normalize: acc*2^-16 (ACT Identity applies
                # the affine scale; Copy does NOT on silicon) times 2^16/den
                # in fp16 (safe: min den ~1.9) -> DVE multiply in 2x mode
                recb = work.tile([D, SL], F16, tag="recb", bufs=2,
                                 name=f"recb{h}")
                nc.scalar.activation(recb[:], lnb[:], AF.Exp, scale=-1.0,
                                     bias=bias16[0:D, 0:1])
                acc_sb = work.tile([D, SL], F16, tag="accsb", bufs=2,
                                   name=f"accsb{h}")
                nc.scalar.activation(acc_sb[:], acc[0:D, :], AF.Identity,
                                     scale=float(2.0 ** -16))
                hsl = hT[h // 2][(h % 2) * D:(h % 2) * D + D, :]
                nc.vector.tensor_mul(hsl, acc_sb[:], recb[:])
                if h == H - 1:
                    # critical path into the gather: elu(x) =
                    # (min(exp(x),1) - 1) + max(x,0) -- h values are < ~4
                    # so exp(x) cannot overflow fp16
                    texp = work.tile([D, SL], F16, tag="texp", bufs=2,
                                     name=f"texp{h}")
                    nc.scalar.activation(texp[:], hsl, AF.Exp)
                    t1 = work.tile([D, SL], F16, tag="tlin", bufs=2,
                                   name=f"t1{h}")
                    nc.vector.tensor_scalar(t1[:], texp[:], 1.0, -1.0,
                                            A.min, A.add)
                    tl = work.tile([D, SL], F16, tag="a1", bufs=2,
                                   name=f"tl{h}")
                    nc.vector.tensor_scalar_max(tl[:], hsl, 0.0)
                    nc.vector.tensor_add(hsl, t1[:], tl[:])
                else:
                    # elu(x) = (x max 0 - 1) + exp(-relu(-x)), ACT-heavy
                    a1 = work.tile([D, SL], F16, tag="a1", bufs=2,
                                   name=f"a1{h}")
                    nc.scalar.activation(a1[:], hsl, AF.Relu, scale=-1.0)
                    texp = work.tile([D, SL], F16, tag="texp", bufs=2,
                                     name=f"texp{h}")
                    nc.scalar.activation(texp[:], a1[:], AF.Exp, scale=-1.0)
                    tlin = work.tile([D, SL], F16, tag="tlin", bufs=2,
                                     name=f"tlin{h}")
                    nc.vector.tensor_scalar(tlin[:], hsl, 0.0, -1.0, A.max,
                                            A.add)
                    nc.vector.tensor_add(hsl, texp[:], tlin[:])
            return emit

        pending = None
        for h in range(H):
            acc = psum.tile([D + 1, SL], F32, tag="acc", bufs=4,
                            name=f"acc{h}")
            for qq in range(NPAIR):
                if qq == 1 and h + 1 < H:
                    emit_g(h + 1)
                if h < 4:
                    t0 = h * 8 + 2 * qq
                    for t in (t0, t0 + 1):
                        emit_stage_mm_copy(t, 1, on_dve=False)
                    emit_extract(t0, 2, 1)
                rk, ek = [], []
                for two in range(2):
                    rr, ee = [], []
                    for k in range(4):
                        r_ap, e_ap = scal(h, 8 * qq + 4 * two + k)
                        rr.append(r_ap)
                        ee.append(e_ap)
                    rk.append(rr)
                    ek.append(ee)
                mq = emit_masked_pair(f"{h}_{qq}", g_bc[h][:], rk, ek, qq)
                for mi, (two, k) in enumerate(MM_ORDER):
                    nc.tensor.matmul(
                        acc[:], lhst(h, 8 * qq + 4 * two + k),
                        mq[:, (4 * two + k) * SL:(4 * two + k + 1) * SL],
                        start=(qq == 0 and mi == 0),
                        stop=(qq == NPAIR - 1 and mi == 7))
                if qq == 0 and pending is not None:
                    pending()
                    pending = None
                    if h % 2 == 0 and h > 1:
                        emit_wh2_part(h // 2 - 1)
            pending = make_epilogue(h, acc)
        pending()

        # ---- last wh2 part + exchange staging: the kt=3 accumulation adds
        # straight into the fp8 p2 tile (skips a separate convert copy) ----
        cc_in = dram.tile([SL, 66], F8, tag="cc_in", name="cc_in")
        cc_full = dram.tile([N, 66], F8, tag="cc_full", addr_space="Shared",
                            name="cc_full")
        p2_sb = work.tile([P, 264], F8, tag="p2", name="p2")
        o3_ps = psum.tile([P, 264], F32, tag="bank", bufs=4, name="o2p3")
        for sub in range(4):
            nc.tensor.matmul(o3_ps[:, sub * 66:(sub + 1) * 66],
                             hT[3][:, sub * P:(sub + 1) * P],
                             woext_all[:, 3 * 66:4 * 66],
                             start=True, stop=True)
        nc.vector.tensor_add(p2_sb[:], wh2acc[:], o3_ps[:])
        nc.sync.dma_start(
            out=cc_in[:].rearrange("(k p) c -> p k c", k=4),
            in_=p2_sb[:].rearrange("p (k c) -> p k c", k=4))
        nc.gpsimd.collective_compute(
            "AllGather", A.bypass, ins=[cc_in[:]], outs=[cc_full[:]],
            replica_groups=[list(range(N_CORES))])

        # f_src2 + g2 prep executes inside the collective's idle window (it
        # only feeds the layer-2 TSPs); its 1x PSUM adds used to burn
        # mid-phase DVE time. Pure reorder: hT tiles are immutable by now.
        for kt in range(NKF):
            fr2_ps = psum.tile([1, SL], F32, tag="bank", bufs=4,
                               name=f"fr2p{kt}")
            nc.tensor.matmul(fr2_ps[:],
                             woext_all[:, kt * 66 + 64:kt * 66 + 65],
                             hT[kt][:], start=True, stop=True)
            if kt == 0:
                nc.scalar.activation(fr2acc[:], fr2_ps[:], AF.Copy)
            else:
                nc.vector.tensor_add(fr2acc[:], fr2acc[:], fr2_ps[:])
        bc2_ps = psum.tile([P, SL], F32, tag="bank", bufs=4, name="gbc2")
        nc.tensor.matmul(bc2_ps[:], ones1[0:1, :], fr2acc[:], start=True,
                         stop=True)
        g2 = persist.tile([P, SL], F16, tag="g2", name="g2")
        nc.scalar.activation(g2[:], bc2_ps[:], AF.Exp, scale=0.8)

        # ---- layer 2: quarter loads (one per quad-pair); e1/r extracted
        # straight from the fp8 tile and lhsT cols converted per quarter so
        # the first TSPs never wait on the full convert. The denominator
        # ones-columns are memset BEFORE the collective (convert skips
        # them), taking that off the post-gather critical path. ----------
        cc_all = persist.tile([P, NQ * 264], F16, tag="cc_all", name="cc_all")
        cc_raw = persist.tile([P, NQ * 264], F8, tag="cc_raw", name="cc_raw")
        nc.vector.memset(cc_all[:, 64:NQ * 264:66], 1.0)
        e1x = [persist.tile([P, 8], F32, tag=f"e2_{qt}", name=f"e2_{qt}")
               for qt in range(NPAIR)]
        rx = [persist.tile([P, 8], F32, tag=f"r2_{qt}", name=f"r2_{qt}")
              for qt in range(NPAIR)]
        for qt in range(NPAIR):
            sl8 = slice(qt * 528, (qt + 1) * 528)
            nc.sync.dma_start(
                out=cc_raw[:, sl8].rearrange("p (g k c) -> p g k c", g=2,
                                             k=4),
                in_=cc_full[qt * N // 4:(qt + 1) * N // 4, :].rearrange(
                    "(g k p) c -> p g k c", g=2, k=4))
            raw8 = cc_raw[:, sl8].rearrange("p (g c) -> p g c", g=8)
            nc.scalar.activation(e1x[qt][:], raw8[:, :, 65:66], AF.Exp)
            nc.scalar.activation(rx[qt][:], raw8[:, :, 65:66], AF.Exp,
                                 scale=-0.8)
            nc.scalar.activation(
                cc_all[:, sl8].rearrange("p (g c) -> p g c", g=8)[:, :, 0:64],
                raw8[:, :, 0:64], AF.Copy)

        cc_gp = [cc_all[:, g * 264:(g + 1) * 264] for g in range(NQ)]
        acc2 = psum.tile([D + 1, SL], F32, tag="acc", bufs=4, name="acc2")
        for qq in range(NPAIR):
            mq = emit_masked_pair(
                f"L2_{qq}", g2[:],
                [[rx[qq][:, 4 * two + k:4 * two + k + 1] for k in range(4)]
                 for two in range(2)],
                [[e1x[qq][:, 4 * two + k:4 * two + k + 1] for k in range(4)]
                 for two in range(2)], qq, l2=True)
            for mi, (two, k) in enumerate(MM_ORDER):
                nc.tensor.matmul(
                    acc2[:],
                    cc_gp[2 * qq + two][:, k * 66:k * 66 + 65],
                    mq[:, (4 * two + k) * SL:(4 * two + k + 1) * SL],
                    start=(qq == 0 and mi == 0),
                    stop=(qq == NPAIR - 1 and mi == 7))
        # final normalize + ELU pipelined in two i-halves so ACT and DVE
        # overlap. Scaled-fp16 normalize (Identity applies the affine
        # scale) gets the DVE ops into 2x/4x mode; L2 denominators are
        # O(100+) so 2^16/den is far inside fp16 range. fin is assembled
        # in fp32 only at the last add.
        o2 = persist.tile([D, SL], F16, tag="o2", name="o2")
        fin = persist.tile([D, SL], F32, tag="fin", name="fin")
        for hf in range(2):
            sl2 = slice(hf * SL // 2, (hf + 1) * SL // 2)
            den2 = work.tile([1, SL // 2], F32, tag="den", bufs=2,
                             name=f"den2_{hf}")
            nc.scalar.activation(den2[:], acc2[D:D + 1, sl2], AF.Copy)
            db2_ps = psum.tile([D, SL // 2], F32, tag="bank", bufs=4,
                               name=f"dbc2_{hf}")
            nc.tensor.matmul(db2_ps[:], ones1[0:1, 0:D], den2[:],
                             start=True, stop=True)
            ln2 = work.tile([D, SL // 2], F32, tag="lnb", bufs=2,
                            name=f"ln2_{hf}")
            nc.scalar.activation(ln2[:], db2_ps[:], AF.Ln)
            rec2 = work.tile([D, SL // 2], F16, tag="recb", bufs=2,
                             name=f"rec2_{hf}")
            nc.scalar.activation(rec2[:], ln2[:], AF.Exp, scale=-1.0,
                                 bias=bias16[0:D, 0:1])
            ac2_sb = work.tile([D, SL // 2], F16, tag="accsb", bufs=2,
                               name=f"ac2sb{hf}")
            nc.scalar.activation(ac2_sb[:], acc2[0:D, sl2], AF.Identity,
                                 scale=float(2.0 ** -16))
            nc.vector.tensor_mul(o2[:, sl2], ac2_sb[:], rec2[:])
            # elu(y) = (min(exp(y),1) - 1) + max(y,0); y < ~4 so exp safe
            t2exp = work.tile([D, SL // 2], F16, tag="t2exp", bufs=2,
                              name=f"t2exp{hf}")
            nc.scalar.activation(t2exp[:], o2[:, sl2], AF.Exp)
            t2a = work.tile([D, SL // 2], F16, tag="t2lin", bufs=2,
                            name=f"t2a{hf}")
            nc.vector.tensor_scalar(t2a[:], t2exp[:], 1.0, -1.0, A.min,
                                    A.add)
            t2b = work.tile([D, SL // 2], F16, tag="a2", bufs=2,
                            name=f"t2b{hf}")
            nc.vector.tensor_scalar_max(t2b[:], o2[:, sl2], 0.0)
            nc.vector.tensor_add(fin[:, sl2], t2a[:], t2b[:])
            nc.sync.dma_start(out=outT[:, sl2], in_=fin[:, sl2])

    nc.compile()
    return nc


# ---------------------------------------------------------------------------
# host-side driver
# ---------------------------------------------------------------------------

def _prep_inputs(x, adj, W, a, Wo, ao):
    xT16 = np.ascontiguousarray(x.T.astype(np.float16))
    adjT = adj.T
    wext = np.empty((F, H * 66), np.float32)
    for h in range(H):
        wext[:, h * 66:h * 66 + D] = W[h]
        wext[:, h * 66 + D] = W[h] @ a[h, :D]
        wext[:, h * 66 + D + 1] = W[h] @ a[h, D:]
    wext = wext.astype(np.float16)
    woext = np.concatenate(
        [Wo, (Wo @ ao[:D])[:, None], (Wo @ ao[D:])[:, None]],
        axis=1).astype(np.float16)

    from ml_dtypes import float8_e4m3fn as f8dt

    in_maps = []
    for c in range(N_CORES):
        sl = slice(c * SL, (c + 1) * SL)
        adjq = np.empty((NQ * P, 4 * SL), np.float32)
        for q in range(NQ):
            for k in range(4):
                jt = 4 * q + k
                adjq[q * P:(q + 1) * P, k * SL:(k + 1) * SL] = \
                    adjT[jt * P:(jt + 1) * P, sl]
        selDh = np.zeros((H, H * P), np.float16)
        for h in range(H):
            selDh[h, h * P:(h + 1) * P] = 1.0
        in_maps.append({
            "xT": xT16,
            "xS": np.ascontiguousarray(xT16[:, sl]),
            "adjP": adjq[:, :SPOOL].astype(f8dt),
            "adjD": adjq[:, SPOOL:].astype(np.float16),
            "Wext": wext,
            "Woext": woext,
            "selD": selDh,
        })
    return in_maps


def kernel(x, adj, W, a, Wo, ao, cfg):
    x = np.asarray(x, np.float32)
    adj = np.asarray(adj, np.float32)
    W = np.asarray(W, np.float32)
    a = np.asarray(a, np.float32)
    Wo = np.asarray(Wo, np.float32)
    ao = np.asarray(ao, np.float32)

    in_maps = _prep_inputs(x, adj, W, a, Wo, ao)
    if _CACHED.get("nc") is None:
        _CACHED["nc"] = build_kernel()
    res = run_bass_kernel_spmd(_CACHED["nc"], in_maps,
                               core_ids=list(range(N_CORES)))
    out = np.empty((N, D), np.float32)
    for c in range(N_CORES):
        out[c * SL:(c + 1) * SL, :] = res.results[c]["outT"].T
    return out


if __name__ == "__main__":
    import reference as ref_mod
    inputs = {k: np.asarray(v) for k, v in ref_mod.setup_inputs().items()}
    expected = np.asarray(ref_mod.reference(**ref_mod.setup_inputs()))
    got = kernel(**inputs)
    err = np.abs(got - expected).max() / np.abs(expected).max()
    print("rel err:", err)
